# revision 13
# baseline (speedup 1.0000x reference)
"""Wilson-Dirac operator on Trainium2, 8 NeuronCores, T-axis domain decomposition.

v3: all-f16 compute in PLANAR (SoA) layout — every SBUF tile is
[component-plane][z*t] with the site dimension contiguous innermost, so all
hot DVE ops stream long 144-element runs (measured: short strided runs cost
~7-8 cycles each; planar removes them). DVE 2x packed f16 mode throughout.

Host arrays per core (f16, gauge pre-scaled by -0.5, fwd links pre-rolled,
all component-plane-major per lattice row):
  psi_h [XY, 24, Z+2, TS+2]  psi slab, z+t halos, planes (s,p,c)
  fi4   [4, XY, 24, Z*TS]    interior psi pre-rolled (x+1, x-1, y+1, y-1)
  WF/WB [4, XY, 18, Z*TS]    hopping matrices, planes (p, A, B)
  outp  [XY, 24, Z*TS]       output, planes (s,p,c)

Blocks: 4 x (128 rows, z 0..24) + 1 x (64 rows, z halved across partitions).
Per (mu,sgn): proj h[j,p,b] -> products P[j,g,A,B] (h broadcast on outer A)
-> bsum over B (add1 on gpsimd, software-pipelined one term) -> combine
m[j,p,a] -> expand. Mass on ACT. Loads on SP queue, stores on ACT queue.
"""

import numpy as np

# ---------------------------------------------------------------- constants
X = Y = Z = 24
T = 48
NCORES = 8
TS = T // NCORES
TH = TS + 2
XY = X * Y
MASSP4 = 4.5

DIRSPEC = {
    0: dict(B=(3, 2), c=(-1j, -1j), e=(1, 0), d=(+1j, +1j)),
    1: dict(B=(3, 2), c=(-1, +1),   e=(1, 0), d=(+1, -1)),
    2: dict(B=(2, 3), c=(-1j, +1j), e=(0, 1), d=(+1j, -1j)),
    3: dict(B=(2, 3), c=(+1, +1),   e=(0, 1), d=(+1, +1)),
}

_CACHE = {}


def _split_waits_json(raw: bytes) -> bytes:
    """Walrus allows only ONE sync-wait per instruction: hoist extras onto
    NoOps inserted immediately before (same engine; sems monotonic => exact)."""
    import json
    bj = json.loads(raw)
    nid = 0
    for fn in bj.get("functions", []):
        for bb in fn.get("blocks", []):
            out = []
            changed = False
            for inst in bb.get("instructions", []):
                si = inst.get("sync_info")
                ow = (si or {}).get("on_wait") or []
                if len(ow) > 1:
                    changed = True
                    for w in ow[:-1]:
                        nid += 1
                        out.append({
                            "engine": inst["engine"], "ins": [], "outs": [],
                            "name": f"WSPL-{nid}", "opcode": "NoOp",
                            "sync_info": {"on_update": [], "on_wait": [w]},
                        })
                    si["on_wait"] = [ow[-1]]
                out.append(inst)
            if changed:
                bb["instructions"] = out
    return json.dumps(bj).encode()


def _install_json_wait_fix():
    import concourse.bass as bass
    if getattr(bass.Bass, "_wd_wait_fix", False):
        return
    orig = bass.Bass.to_json_bytes

    def patched(self, *a, **k):
        return _split_waits_json(orig(self, *a, **k))

    bass.Bass.to_json_bytes = patched
    bass.Bass._wd_wait_fix = True


def build_module(pool_add1=False):
    import concourse.bass as bass
    import concourse.mybir as mybir
    from concourse.ap import AP
    from concourse.mybir import AluOpType
    from concourse.tile import TileContext

    _install_json_wait_fix()
    F16 = mybir.dt.float16

    nc = bass.Bass()
    psi_h = nc.declare_dram_parameter("psi_h", [XY, 24, (Z + 2) * TH], F16, isOutput=False)
    fi4 = nc.declare_dram_parameter("fi4", [4, XY, 24, Z * TS], F16, isOutput=False)
    WFp = nc.declare_dram_parameter("WF", [4, XY, 27, Z * TS], F16, isOutput=False)
    WBp = nc.declare_dram_parameter("WB", [4, XY, 27, Z * TS], F16, isOutput=False)
    outp = nc.declare_dram_parameter("outp", [XY, 24, Z * TS], F16, isOutput=True)

    blocks = [(0, 128, [(0, 24, 0)]), (128, 128, [(0, 24, 0)]),
              (256, 128, [(0, 24, 0)]), (384, 128, [(0, 24, 0)]),
              (512, 64, [(0, 12, 0), (12, 12, 64)])]

    def sap(t, off, dims):
        return AP(t.tensor, t.offset + off, [list(t.ap[0])] + [list(d) for d in dims])

    with TileContext(nc) as tc:
        ctx_pool = tc.tile_pool(name="work", bufs=1)
        pool = ctx_pool.__enter__()
        V = nc.vector
        G = nc.gpsimd
        D = nc.sync
        ZFULL = Z * TS
        for (r0, R, parts) in blocks:
            zh = parts[0][1]
            npart = R * len(parts)
            ZT = zh * TS          # sites per row-slice
            PS = (zh + 2) * TH    # psi_al plane stride

            psi_al = pool.tile([npart, 24 * PS], F16, tag="psi_al", bufs=3)
            out_t = pool.tile([npart, 12 * ZT], F16, tag="out_t", bufs=2)
            out23 = pool.tile([npart, 12 * ZT], F16, tag="out23", bufs=2)
            # psi_h planar per-plane z-slice load (full-z: whole row)
            for (z0, _, p0) in parts:
                D.dma_start(out=psi_al[p0:p0 + R],
                            in_=psi_h[r0:r0 + R, :, z0 * TH:(z0 + zh + 2) * TH])

            def load(tag, src, mu, bufs, ncp):
                tl = pool.tile([npart, ncp * ZT], F16, tag=tag, bufs=bufs)
                for (z0, _, p0) in parts:
                    D.dma_start(out=tl[p0:p0 + R],
                                in_=src[mu, r0:r0 + R, :, z0 * TS:(z0 + zh) * TS])
                return tl

            # mass on ACT: out = 4.5 * psi interior (planes, z, t)
            nc.scalar.mul(
                sap(out_t, 0, [[ZT, 12], [6, zh], [1, 6]]),
                sap(psi_al, TH + 1, [[PS, 12], [TH, zh], [1, 6]]),
                MASSP4)
            nc.scalar.mul(
                sap(out23, 0, [[ZT, 12], [6, zh], [1, 6]]),
                sap(psi_al, 12 * PS + TH + 1, [[PS, 12], [TH, zh], [1, 6]]),
                MASSP4)

            tail_q = []

            EX = V
            def run_tail():
                if not tail_q:
                    return
                pt, st, mt, spec, dj = tail_q.pop()
                # bsum part 2 (add1 ran on gpsimd one term ago)
                V.tensor_tensor(sap(st, 0, [[ZT, 18], [1, ZT]]),
                                sap(st, 0, [[ZT, 18], [1, ZT]]),
                                sap(pt, 2 * ZT, [[3 * ZT, 18], [1, ZT]]),
                                AluOpType.add)
                # combine: m_re = S1-S2; tmp(m_im) = S1+S2; m_im = S5-tmp
                V.tensor_tensor(sap(mt, 0, [[6 * ZT, 2], [ZT, 3], [1, ZT]]),
                                sap(st, 0, [[9 * ZT, 2], [ZT, 3], [1, ZT]]),
                                sap(st, 3 * ZT, [[9 * ZT, 2], [ZT, 3], [1, ZT]]),
                                AluOpType.subtract)
                V.tensor_tensor(sap(mt, 3 * ZT, [[6 * ZT, 2], [ZT, 3], [1, ZT]]),
                                sap(st, 0, [[9 * ZT, 2], [ZT, 3], [1, ZT]]),
                                sap(st, 3 * ZT, [[9 * ZT, 2], [ZT, 3], [1, ZT]]),
                                AluOpType.add)
                V.tensor_tensor(sap(mt, 3 * ZT, [[6 * ZT, 2], [ZT, 3], [1, ZT]]),
                                sap(st, 6 * ZT, [[9 * ZT, 2], [ZT, 3], [1, ZT]]),
                                sap(mt, 3 * ZT, [[6 * ZT, 2], [ZT, 3], [1, ZT]]),
                                AluOpType.subtract)
                # expand: s01 on DVE (out_t), s23 on gpsimd (out23)
                o01 = sap(out_t, 0, [[ZT, 12], [1, ZT]])
                V.tensor_tensor(o01, o01, sap(mt, 0, [[ZT, 12], [1, ZT]]),
                                AluOpType.add)
                d0, d1 = dj
                if d0.imag == 0.0:
                    if d0.real == d1.real and spec["e"][0] == 0:
                        o23 = sap(out23, 0, [[ZT, 12], [1, ZT]])
                        G.tensor_tensor(o23, o23, sap(mt, 0, [[ZT, 12], [1, ZT]]),
                                        AluOpType.add if d0.real > 0 else AluOpType.subtract)
                    else:
                        for si, (e, dv) in enumerate(zip(spec["e"], dj)):
                            os_ = sap(out23, 6 * si * ZT, [[ZT, 6], [1, ZT]])
                            G.tensor_tensor(os_, os_, sap(mt, e * 6 * ZT, [[ZT, 6], [1, ZT]]),
                                            AluOpType.add if dv.real > 0 else AluOpType.subtract)
                else:
                    for si, (e, dv) in enumerate(zip(spec["e"], dj)):
                        sg = dv.imag > 0
                        ore = sap(out23, 6 * si * ZT, [[ZT, 3], [1, ZT]])
                        G.tensor_tensor(ore, ore,
                                        sap(mt, (e * 6 + 3) * ZT, [[ZT, 3], [1, ZT]]),
                                        AluOpType.subtract if sg else AluOpType.add)
                        oim = sap(out23, (6 * si + 3) * ZT, [[ZT, 3], [1, ZT]])
                        G.tensor_tensor(oim, oim,
                                        sap(mt, e * 6 * ZT, [[ZT, 3], [1, ZT]]),
                                        AluOpType.add if sg else AluOpType.subtract)

            for mu in range(4):
                wf_t = load("w", WFp, mu, 5, 27)
                wb_t = load("w", WBp, mu, 5, 27)
                if mu <= 1:
                    pf_t = load("fi", fi4, 2 * mu, 6, 24)
                    pb_t = load("fi", fi4, 2 * mu + 1, 6, 24)
                spec = DIRSPEC[mu]

                for sgn in (+1, -1):
                    fwd = sgn > 0
                    cj = spec["c"] if fwd else tuple(-v for v in spec["c"])
                    dj = spec["d"] if fwd else tuple(-v for v in spec["d"])
                    wt = wf_t if fwd else wb_t

                    # psi source: planar planes; psi_al has (z,t) halo dims
                    if mu <= 1:
                        ps = pf_t if fwd else pb_t
                        pbase, pstr, pz = 0, ZT, [[1, ZT]]
                        hz = [[1, ZT]]
                    else:
                        if mu == 2:
                            pbase = (0 if fwd else 2 * TH) + 1
                        else:
                            pbase = TH + (0 if fwd else 2)
                        ps, pstr, pz = psi_al, PS, [[TH, zh], [1, 6]]
                        hz = [[6, zh], [1, 6]]

                    # --- proj h[j,p,b] = psi[A] + c*psi[B]  (plane-major)
                    ht = pool.tile([npart, 18 * ZT], F16, tag="h", bufs=2)
                    for j in (0, 1):
                        A, B, c = j, spec["B"][j], cj[j]
                        if c.imag == 0.0:
                            op = AluOpType.add if c.real > 0 else AluOpType.subtract
                            V.tensor_tensor(
                                sap(ht, j * 9 * ZT, [[ZT, 6]] + hz),
                                sap(ps, pbase + A * 6 * pstr, [[pstr, 6]] + pz),
                                sap(ps, pbase + B * 6 * pstr, [[pstr, 6]] + pz), op)
                        else:
                            sg = c.imag > 0
                            V.tensor_tensor(
                                sap(ht, j * 9 * ZT, [[ZT, 3]] + hz),
                                sap(ps, pbase + A * 6 * pstr, [[pstr, 3]] + pz),
                                sap(ps, pbase + (B * 6 + 3) * pstr, [[pstr, 3]] + pz),
                                AluOpType.subtract if sg else AluOpType.add)
                            V.tensor_tensor(
                                sap(ht, (j * 9 + 3) * ZT, [[ZT, 3]] + hz),
                                sap(ps, pbase + (A * 6 + 3) * pstr, [[pstr, 3]] + pz),
                                sap(ps, pbase + B * 6 * pstr, [[pstr, 3]] + pz),
                                AluOpType.add if sg else AluOpType.subtract)

                    # --- hsum: h[j,sum,b] = h[j,re,b] + h[j,im,b]
                    V.tensor_tensor(sap(ht, 6 * ZT, [[9 * ZT, 2], [ZT, 3], [1, ZT]]),
                                    sap(ht, 0, [[9 * ZT, 2], [ZT, 3], [1, ZT]]),
                                    sap(ht, 3 * ZT, [[9 * ZT, 2], [ZT, 3], [1, ZT]]),
                                    AluOpType.add)
                    # --- Karatsuba products P[j,k,A,B] = W[k,A,B] * h[j,k,B]
                    # k: (re*hre, im*him, sum*hsum)
                    pt = pool.tile([npart, 54 * ZT], F16, tag="P", bufs=2)
                    for j in (0, 1):
                        for k in range(3):
                            V.tensor_tensor(
                                sap(pt, (j * 27 + k * 9) * ZT, [[3 * ZT, 3], [ZT, 3], [1, ZT]]),
                                sap(wt, k * 9 * ZT, [[3 * ZT, 3], [ZT, 3], [1, ZT]]),
                                sap(ht, (j * 9 + k * 3) * ZT, [[0, 3], [ZT, 3], [1, ZT]]),
                                AluOpType.mult)

                    # --- bsum part 1: S = P[B0] + P[B1]
                    st = pool.tile([npart, 18 * ZT], F16, tag="S", bufs=3)
                    V.tensor_tensor(sap(st, 0, [[ZT, 18], [1, ZT]]),
                                    sap(pt, 0, [[3 * ZT, 18], [1, ZT]]),
                                    sap(pt, ZT, [[3 * ZT, 18], [1, ZT]]),
                                    AluOpType.add)
                    mt = pool.tile([npart, 12 * ZT], F16, tag="m", bufs=6)

                    run_tail()
                    tail_q.append((pt, st, mt, spec, dj))

            run_tail()
            for (z0, _, p0) in parts:
                nc.scalar.dma_start(out=outp[r0:r0 + R, 0:12, z0 * TS:(z0 + zh) * TS],
                                    in_=out_t[p0:p0 + R])
                nc.scalar.dma_start(out=outp[r0:r0 + R, 12:24, z0 * TS:(z0 + zh) * TS],
                                    in_=out23[p0:p0 + R])
        ctx_pool.__exit__(None, None, None)
    return nc


# ---------------------------------------------------------------- host side
def prep_core_inputs(field, gauge, t0):
    """field [X,Y,Z,T,3,4] c64, gauge [4,X,Y,Z,T,3,3] c64 -> planar f16."""
    tsl = [(t0 + i) % T for i in range(TS)]
    th_idx = [(t0 - 1) % T] + tsl + [(t0 + TS) % T]
    f = field[:, :, :, th_idx]
    fr = np.stack([f.real, f.imag], axis=-1)            # [X,Y,Z,TH,c,s,p]
    fpl = fr.transpose(0, 1, 5, 6, 4, 2, 3)             # [X,Y,s,p,c,Z,TH]
    zhal = np.concatenate([fpl[..., -1:, :], fpl, fpl[..., :1, :]], axis=5)
    psi_h = np.ascontiguousarray(zhal).reshape(XY, 24 * (Z + 2) * TH).astype(np.float16)

    fin = fpl[..., :, 1:TS + 1]                         # [X,Y,s,p,c,Z,TS]
    rolls = [np.roll(fin, +1, 0), np.roll(fin, -1, 0),
             np.roll(fin, +1, 1), np.roll(fin, -1, 1)]
    fi4 = np.stack([np.ascontiguousarray(r).reshape(XY, 24 * Z * TS) for r in rolls]
                   ).astype(np.float16)

    WF = np.empty((4, XY, 27 * Z * TS), np.float16)
    WB = np.empty((4, XY, 27 * Z * TS), np.float16)
    for mu in range(4):
        Ub = gauge[mu][:, :, :, tsl]                    # [X,Y,Z,TS,A,B]
        vb = np.stack([Ub.real, Ub.imag, Ub.real + Ub.imag], axis=4) * np.float32(-0.5)
        vbp = vb.transpose(0, 1, 4, 5, 6, 2, 3)         # [X,Y,k,A,B,Z,TS]
        WB[mu] = np.ascontiguousarray(vbp).reshape(XY, 27 * Z * TS).astype(np.float16)
        if mu == 3:
            tf = [(t0 - 1 + i) % T for i in range(TS)]
            Uf = gauge[mu][:, :, :, tf]
        else:
            Uf = np.roll(gauge[mu], +1, axis=mu)[:, :, :, tsl]
        Vf = np.conjugate(np.swapaxes(Uf, -1, -2))
        vf = np.stack([Vf.real, Vf.imag, Vf.real + Vf.imag], axis=4) * np.float32(-0.5)
        vfp = vf.transpose(0, 1, 4, 5, 6, 2, 3)
        WF[mu] = np.ascontiguousarray(vfp).reshape(XY, 27 * Z * TS).astype(np.float16)
    return {"psi_h": psi_h, "fi4": fi4, "WF": WF, "WB": WB}


def prep_in_maps(field, gauge):
    return [prep_core_inputs(field, gauge, k * TS) for k in range(NCORES)]


def assemble_output(res):
    out = np.empty((X, Y, Z, T, 3, 4), np.complex64)
    for k in range(NCORES):
        o = res[k]["outp"].reshape(X, Y, 4, 2, 3, Z, TS).astype(np.float32)
        oc = (o[:, :, :, 0] + 1j * o[:, :, :, 1])       # [X,Y,s,c,Z,TS]
        out[:, :, :, k * TS:(k + 1) * TS] = oc.transpose(0, 1, 4, 5, 3, 2)
    return out


def kernel(field, gauge_field):
    from concourse.bass_utils import run_bass_kernel_spmd

    if "v3" not in _CACHE:
        _CACHE["v3"] = build_module()
    nc = _CACHE["v3"]
    in_maps = prep_in_maps(np.asarray(field), np.asarray(gauge_field))
    res = run_bass_kernel_spmd(nc, in_maps, list(range(NCORES))).results
    return assemble_output(res)


# revision 14
# speedup vs baseline: 1.1645x; 1.1645x over previous
"""Wilson-Dirac operator on Trainium2, 8 NeuronCores, T-axis domain decomposition.

v3: all-f16 compute in PLANAR (SoA) layout — every SBUF tile is
[component-plane][z*t] with the site dimension contiguous innermost, so all
hot DVE ops stream long 144-element runs (measured: short strided runs cost
~7-8 cycles each; planar removes them). DVE 2x packed f16 mode throughout.

Host arrays per core (f16, gauge pre-scaled by -0.5, fwd links pre-rolled,
all component-plane-major per lattice row):
  psi_h [XY, 24, Z+2, TS+2]  psi slab, z+t halos, planes (s,p,c)
  fi4   [4, XY, 24, Z*TS]    interior psi pre-rolled (x+1, x-1, y+1, y-1)
  WF/WB [4, XY, 18, Z*TS]    hopping matrices, planes (p, A, B)
  outp  [XY, 24, Z*TS]       output, planes (s,p,c)

Blocks: 4 x (128 rows, z 0..24) + 1 x (64 rows, z halved across partitions).
Per (mu,sgn): proj h[j,p,b] -> products P[j,g,A,B] (h broadcast on outer A)
-> bsum over B (add1 on gpsimd, software-pipelined one term) -> combine
m[j,p,a] -> expand. Mass on ACT. Loads on SP queue, stores on ACT queue.
"""

import numpy as np

# ---------------------------------------------------------------- constants
X = Y = Z = 24
T = 48
NCORES = 8
TS = T // NCORES
TH = TS + 2
XY = X * Y
MASSP4 = 4.5

DIRSPEC = {
    0: dict(B=(3, 2), c=(-1j, -1j), e=(1, 0), d=(+1j, +1j)),
    1: dict(B=(3, 2), c=(-1, +1),   e=(1, 0), d=(+1, -1)),
    2: dict(B=(2, 3), c=(-1j, +1j), e=(0, 1), d=(+1j, -1j)),
    3: dict(B=(2, 3), c=(+1, +1),   e=(0, 1), d=(+1, +1)),
}

_CACHE = {}


def _split_waits_json(raw: bytes) -> bytes:
    """Walrus allows only ONE sync-wait per instruction: hoist extras onto
    NoOps inserted immediately before (same engine; sems monotonic => exact)."""
    import json
    bj = json.loads(raw)
    nid = 0
    for fn in bj.get("functions", []):
        for bb in fn.get("blocks", []):
            out = []
            changed = False
            for inst in bb.get("instructions", []):
                si = inst.get("sync_info")
                ow = (si or {}).get("on_wait") or []
                if len(ow) > 1:
                    changed = True
                    for w in ow[:-1]:
                        nid += 1
                        out.append({
                            "engine": inst["engine"], "ins": [], "outs": [],
                            "name": f"WSPL-{nid}", "opcode": "NoOp",
                            "sync_info": {"on_update": [], "on_wait": [w]},
                        })
                    si["on_wait"] = [ow[-1]]
                out.append(inst)
            if changed:
                bb["instructions"] = out
    return json.dumps(bj).encode()


def _install_json_wait_fix():
    import concourse.bass as bass
    if getattr(bass.Bass, "_wd_wait_fix", False):
        return
    orig = bass.Bass.to_json_bytes

    def patched(self, *a, **k):
        return _split_waits_json(orig(self, *a, **k))

    bass.Bass.to_json_bytes = patched
    bass.Bass._wd_wait_fix = True


def build_module(pool_add1=False):
    import concourse.bass as bass
    import concourse.mybir as mybir
    from concourse.ap import AP
    from concourse.mybir import AluOpType
    from concourse.tile import TileContext

    _install_json_wait_fix()
    F16 = mybir.dt.float16

    nc = bass.Bass()
    psi_h = nc.declare_dram_parameter("psi_h", [XY, 24, (Z + 2) * TH], F16, isOutput=False)
    fi4 = nc.declare_dram_parameter("fi4", [4, XY, 24, Z * TS], F16, isOutput=False)
    WFp = nc.declare_dram_parameter("WF", [4, XY, 27, Z * TS], F16, isOutput=False)
    WBp = nc.declare_dram_parameter("WB", [4, XY, 27, Z * TS], F16, isOutput=False)
    outp = nc.declare_dram_parameter("outp", [XY, 24, Z * TS], F16, isOutput=True)

    blocks = [(0, 128, [(0, 24, 0)]), (128, 128, [(0, 24, 0)]),
              (256, 128, [(0, 24, 0)]), (384, 128, [(0, 24, 0)]),
              (512, 64, [(0, 12, 0), (12, 12, 64)])]

    def sap(t, off, dims):
        return AP(t.tensor, t.offset + off, [list(t.ap[0])] + [list(d) for d in dims])

    with TileContext(nc) as tc:
        ctx_pool = tc.tile_pool(name="work", bufs=1)
        pool = ctx_pool.__enter__()
        V = nc.vector
        G = nc.gpsimd
        D = nc.sync
        ZFULL = Z * TS
        for (r0, R, parts) in blocks:
            zh = parts[0][1]
            npart = R * len(parts)
            ZT = zh * TS          # sites per row-slice
            PS = (zh + 2) * TH    # psi_al plane stride

            psi_al = pool.tile([npart, 24 * PS], F16, tag="psi_al", bufs=3)
            out_t = pool.tile([npart, 12 * ZT], F16, tag="out_t", bufs=2)
            out23 = pool.tile([npart, 12 * ZT], F16, tag="out23", bufs=2)
            # psi_h planar per-plane z-slice load (full-z: whole row)
            for (z0, _, p0) in parts:
                D.dma_start(out=psi_al[p0:p0 + R],
                            in_=psi_h[r0:r0 + R, :, z0 * TH:(z0 + zh + 2) * TH])

            def load(tag, src, mu, bufs, ncp):
                tl = pool.tile([npart, ncp * ZT], F16, tag=tag, bufs=bufs)
                for (z0, _, p0) in parts:
                    D.dma_start(out=tl[p0:p0 + R],
                                in_=src[mu, r0:r0 + R, :, z0 * TS:(z0 + zh) * TS])
                return tl

            # mass on ACT: out = 4.5 * psi interior (planes, z, t)
            nc.scalar.mul(
                sap(out_t, 0, [[ZT, 12], [6, zh], [1, 6]]),
                sap(psi_al, TH + 1, [[PS, 12], [TH, zh], [1, 6]]),
                MASSP4)
            nc.scalar.mul(
                sap(out23, 0, [[ZT, 12], [6, zh], [1, 6]]),
                sap(psi_al, 12 * PS + TH + 1, [[PS, 12], [TH, zh], [1, 6]]),
                MASSP4)

            tail_q = []

            EX = V
            def run_tail():
                if not tail_q:
                    return
                pt, st, mt, spec, dj = tail_q.pop()
                # bsum part 2 (add1 ran on gpsimd one term ago)
                V.tensor_tensor(sap(st, 0, [[ZT, 18], [1, ZT]]),
                                sap(st, 0, [[ZT, 18], [1, ZT]]),
                                sap(pt, 2 * ZT, [[3 * ZT, 18], [1, ZT]]),
                                AluOpType.add)
                # combine: m_re = S1-S2; tmp(m_im) = S1+S2; m_im = S5-tmp
                V.tensor_tensor(sap(mt, 0, [[6 * ZT, 2], [ZT, 3], [1, ZT]]),
                                sap(st, 0, [[9 * ZT, 2], [ZT, 3], [1, ZT]]),
                                sap(st, 3 * ZT, [[9 * ZT, 2], [ZT, 3], [1, ZT]]),
                                AluOpType.subtract)
                V.tensor_tensor(sap(mt, 3 * ZT, [[6 * ZT, 2], [ZT, 3], [1, ZT]]),
                                sap(st, 0, [[9 * ZT, 2], [ZT, 3], [1, ZT]]),
                                sap(st, 3 * ZT, [[9 * ZT, 2], [ZT, 3], [1, ZT]]),
                                AluOpType.add)
                V.tensor_tensor(sap(mt, 3 * ZT, [[6 * ZT, 2], [ZT, 3], [1, ZT]]),
                                sap(st, 6 * ZT, [[9 * ZT, 2], [ZT, 3], [1, ZT]]),
                                sap(mt, 3 * ZT, [[6 * ZT, 2], [ZT, 3], [1, ZT]]),
                                AluOpType.subtract)
                # expand: s01 on DVE (out_t), s23 on gpsimd (out23)
                o01 = sap(out_t, 0, [[ZT, 12], [1, ZT]])
                V.tensor_tensor(o01, o01, sap(mt, 0, [[ZT, 12], [1, ZT]]),
                                AluOpType.add)
                d0, d1 = dj
                if d0.imag == 0.0:
                    if d0.real == d1.real and spec["e"][0] == 0:
                        o23 = sap(out23, 0, [[ZT, 12], [1, ZT]])
                        V.tensor_tensor(o23, o23, sap(mt, 0, [[ZT, 12], [1, ZT]]),
                                        AluOpType.add if d0.real > 0 else AluOpType.subtract)
                    else:
                        for si, (e, dv) in enumerate(zip(spec["e"], dj)):
                            os_ = sap(out23, 6 * si * ZT, [[ZT, 6], [1, ZT]])
                            V.tensor_tensor(os_, os_, sap(mt, e * 6 * ZT, [[ZT, 6], [1, ZT]]),
                                            AluOpType.add if dv.real > 0 else AluOpType.subtract)
                else:
                    for si, (e, dv) in enumerate(zip(spec["e"], dj)):
                        sg = dv.imag > 0
                        ore = sap(out23, 6 * si * ZT, [[ZT, 3], [1, ZT]])
                        V.tensor_tensor(ore, ore,
                                        sap(mt, (e * 6 + 3) * ZT, [[ZT, 3], [1, ZT]]),
                                        AluOpType.subtract if sg else AluOpType.add)
                        oim = sap(out23, (6 * si + 3) * ZT, [[ZT, 3], [1, ZT]])
                        V.tensor_tensor(oim, oim,
                                        sap(mt, e * 6 * ZT, [[ZT, 3], [1, ZT]]),
                                        AluOpType.add if sg else AluOpType.subtract)

            for mu in range(4):
                wf_t = load("w", WFp, mu, 5, 27)
                wb_t = load("w", WBp, mu, 5, 27)
                if mu <= 1:
                    pf_t = load("fi", fi4, 2 * mu, 6, 24)
                    pb_t = load("fi", fi4, 2 * mu + 1, 6, 24)
                spec = DIRSPEC[mu]

                for sgn in (+1, -1):
                    fwd = sgn > 0
                    cj = spec["c"] if fwd else tuple(-v for v in spec["c"])
                    dj = spec["d"] if fwd else tuple(-v for v in spec["d"])
                    wt = wf_t if fwd else wb_t

                    # psi source: planar planes; psi_al has (z,t) halo dims
                    if mu <= 1:
                        ps = pf_t if fwd else pb_t
                        pbase, pstr, pz = 0, ZT, [[1, ZT]]
                        hz = [[1, ZT]]
                    else:
                        if mu == 2:
                            pbase = (0 if fwd else 2 * TH) + 1
                        else:
                            pbase = TH + (0 if fwd else 2)
                        ps, pstr, pz = psi_al, PS, [[TH, zh], [1, 6]]
                        hz = [[6, zh], [1, 6]]

                    # --- proj h[j,p,b] = psi[A] + c*psi[B]  (plane-major)
                    ht = pool.tile([npart, 18 * ZT], F16, tag="h", bufs=2)
                    for j in (0, 1):
                        A, B, c = j, spec["B"][j], cj[j]
                        if c.imag == 0.0:
                            op = AluOpType.add if c.real > 0 else AluOpType.subtract
                            V.tensor_tensor(
                                sap(ht, j * 9 * ZT, [[ZT, 6]] + hz),
                                sap(ps, pbase + A * 6 * pstr, [[pstr, 6]] + pz),
                                sap(ps, pbase + B * 6 * pstr, [[pstr, 6]] + pz), op)
                        else:
                            sg = c.imag > 0
                            V.tensor_tensor(
                                sap(ht, j * 9 * ZT, [[ZT, 3]] + hz),
                                sap(ps, pbase + A * 6 * pstr, [[pstr, 3]] + pz),
                                sap(ps, pbase + (B * 6 + 3) * pstr, [[pstr, 3]] + pz),
                                AluOpType.subtract if sg else AluOpType.add)
                            V.tensor_tensor(
                                sap(ht, (j * 9 + 3) * ZT, [[ZT, 3]] + hz),
                                sap(ps, pbase + (A * 6 + 3) * pstr, [[pstr, 3]] + pz),
                                sap(ps, pbase + B * 6 * pstr, [[pstr, 3]] + pz),
                                AluOpType.add if sg else AluOpType.subtract)

                    # --- hsum: h[j,sum,b] = h[j,re,b] + h[j,im,b]
                    V.tensor_tensor(sap(ht, 6 * ZT, [[9 * ZT, 2], [ZT, 3], [1, ZT]]),
                                    sap(ht, 0, [[9 * ZT, 2], [ZT, 3], [1, ZT]]),
                                    sap(ht, 3 * ZT, [[9 * ZT, 2], [ZT, 3], [1, ZT]]),
                                    AluOpType.add)
                    # --- Karatsuba products P[j,k,A,B] = W[k,A,B] * h[j,k,B]
                    # k: (re*hre, im*him, sum*hsum)
                    pt = pool.tile([npart, 54 * ZT], F16, tag="P", bufs=2)
                    for j in (0, 1):
                        for k in range(3):
                            V.tensor_tensor(
                                sap(pt, (j * 27 + k * 9) * ZT, [[3 * ZT, 3], [ZT, 3], [1, ZT]]),
                                sap(wt, k * 9 * ZT, [[3 * ZT, 3], [ZT, 3], [1, ZT]]),
                                sap(ht, (j * 9 + k * 3) * ZT, [[0, 3], [ZT, 3], [1, ZT]]),
                                AluOpType.mult)

                    # --- bsum part 1: S = P[B0] + P[B1]
                    st = pool.tile([npart, 18 * ZT], F16, tag="S", bufs=3)
                    V.tensor_tensor(sap(st, 0, [[ZT, 18], [1, ZT]]),
                                    sap(pt, 0, [[3 * ZT, 18], [1, ZT]]),
                                    sap(pt, ZT, [[3 * ZT, 18], [1, ZT]]),
                                    AluOpType.add)
                    mt = pool.tile([npart, 12 * ZT], F16, tag="m", bufs=6)

                    run_tail()
                    tail_q.append((pt, st, mt, spec, dj))

            run_tail()
            for (z0, _, p0) in parts:
                nc.scalar.dma_start(out=outp[r0:r0 + R, 0:12, z0 * TS:(z0 + zh) * TS],
                                    in_=out_t[p0:p0 + R])
                nc.scalar.dma_start(out=outp[r0:r0 + R, 12:24, z0 * TS:(z0 + zh) * TS],
                                    in_=out23[p0:p0 + R])
        ctx_pool.__exit__(None, None, None)
    return nc


# ---------------------------------------------------------------- host side
def prep_core_inputs(field, gauge, t0):
    """field [X,Y,Z,T,3,4] c64, gauge [4,X,Y,Z,T,3,3] c64 -> planar f16."""
    tsl = [(t0 + i) % T for i in range(TS)]
    th_idx = [(t0 - 1) % T] + tsl + [(t0 + TS) % T]
    f = field[:, :, :, th_idx]
    fr = np.stack([f.real, f.imag], axis=-1)            # [X,Y,Z,TH,c,s,p]
    fpl = fr.transpose(0, 1, 5, 6, 4, 2, 3)             # [X,Y,s,p,c,Z,TH]
    zhal = np.concatenate([fpl[..., -1:, :], fpl, fpl[..., :1, :]], axis=5)
    psi_h = np.ascontiguousarray(zhal).reshape(XY, 24 * (Z + 2) * TH).astype(np.float16)

    fin = fpl[..., :, 1:TS + 1]                         # [X,Y,s,p,c,Z,TS]
    rolls = [np.roll(fin, +1, 0), np.roll(fin, -1, 0),
             np.roll(fin, +1, 1), np.roll(fin, -1, 1)]
    fi4 = np.stack([np.ascontiguousarray(r).reshape(XY, 24 * Z * TS) for r in rolls]
                   ).astype(np.float16)

    WF = np.empty((4, XY, 27 * Z * TS), np.float16)
    WB = np.empty((4, XY, 27 * Z * TS), np.float16)
    for mu in range(4):
        Ub = gauge[mu][:, :, :, tsl]                    # [X,Y,Z,TS,A,B]
        vb = np.stack([Ub.real, Ub.imag, Ub.real + Ub.imag], axis=4) * np.float32(-0.5)
        vbp = vb.transpose(0, 1, 4, 5, 6, 2, 3)         # [X,Y,k,A,B,Z,TS]
        WB[mu] = np.ascontiguousarray(vbp).reshape(XY, 27 * Z * TS).astype(np.float16)
        if mu == 3:
            tf = [(t0 - 1 + i) % T for i in range(TS)]
            Uf = gauge[mu][:, :, :, tf]
        else:
            Uf = np.roll(gauge[mu], +1, axis=mu)[:, :, :, tsl]
        Vf = np.conjugate(np.swapaxes(Uf, -1, -2))
        vf = np.stack([Vf.real, Vf.imag, Vf.real + Vf.imag], axis=4) * np.float32(-0.5)
        vfp = vf.transpose(0, 1, 4, 5, 6, 2, 3)
        WF[mu] = np.ascontiguousarray(vfp).reshape(XY, 27 * Z * TS).astype(np.float16)
    return {"psi_h": psi_h, "fi4": fi4, "WF": WF, "WB": WB}


def prep_in_maps(field, gauge):
    return [prep_core_inputs(field, gauge, k * TS) for k in range(NCORES)]


def assemble_output(res):
    out = np.empty((X, Y, Z, T, 3, 4), np.complex64)
    for k in range(NCORES):
        o = res[k]["outp"].reshape(X, Y, 4, 2, 3, Z, TS).astype(np.float32)
        oc = (o[:, :, :, 0] + 1j * o[:, :, :, 1])       # [X,Y,s,c,Z,TS]
        out[:, :, :, k * TS:(k + 1) * TS] = oc.transpose(0, 1, 4, 5, 3, 2)
    return out


def kernel(field, gauge_field):
    from concourse.bass_utils import run_bass_kernel_spmd

    if "v3" not in _CACHE:
        _CACHE["v3"] = build_module()
    nc = _CACHE["v3"]
    in_maps = prep_in_maps(np.asarray(field), np.asarray(gauge_field))
    res = run_bass_kernel_spmd(nc, in_maps, list(range(NCORES))).results
    return assemble_output(res)
